# revision 21
# baseline (speedup 1.0000x reference)
"""Trainium2 Bass kernel for the non-local attention block (nn_ASM_5196910428634).

8 NeuronCores, data-parallel over batch (1 element per core).  Per core:
  x = fuse[b] as [C=256, HW=4096] (host-cast to fp16: halves the DMA-in
      stream that paces the conv phase; residual in fp16 costs ~2e-3 abs)
  theta = Wt @ x + bt                [128, 4096]   (fp16 conv, fp16 out)
  phi   = pool2(Wp @ x + bp)         [128, 1024]   (fp16 conv, fp16 out)
  g     = pool2(Wg @ x + bg)         [128, 1024]   (fp16 conv, bf16 out)
  fT[k, n] = sum_ci phi[ci, k] theta[ci, n]        (fp16 matmul, k=1024)
  A = exp(fT)  -> bf16               (softmax w/o max-subtraction: |f| << 88,
                                      and bf16 carries the fp32 exponent range)
  sums[n]: 3-level bf16 pairwise tree over A's 8 k-chunks on DVE, then ONE
           all-ones 128x128 stationary matmul for the partition reduction
           (whose output rows ARE the broadcast) - replaces the 8 ones
           matmuls/window of the previous version (-12us PE)
  yT[c, n] = sum_k gT[k, c] A[k, n]  (bf16 matmul, unnormalized)
  y_norm = yT * (1/sums)             (reciprocal_approx_fast on DVE, the
                                      normalize STT on the GpSimd engine)
  z = WW' @ y_norm + x'              (bf16 W conv with the BN scale folded
                                      into WW' on host; x' = x + bias2 is
                                      host-folded with conv biases adjusted
                                      by -W_proj @ bias2, so the residual +
                                      BN bias is ONE identity matmul into
                                      the same psum, and z DMAs straight
                                      from PSUM - no vector-engine tail)
Host assembles out = concat([lc, z, gc], axis=1) (pure pass-through channels).

Schedule notes:
- ~8 dummy 512-row matmuls right after the const memsets keep the PE
  executing through the DMA head so its p-state ramps to 2.4GHz before the
  first conv (cold matmuls otherwise run 2x slow for the first ~3us).
- x + weights ride the sync/scalar HWDGE queues (t=0 rows on sync, t=1 on
  scalar) in conv consumption order; the stream is HBM-bandwidth-bound.
- phi's 2x2 maxpool is split per conv chunk q so window 0's f matmuls and
  exps run interleaved WITH the conv phase; the ScalarE exp stream (the
  steady-state pacer at ~4.5us/window) starts ~13us in.
- steady-state loop per window w: f(w,k2=0,1)/exp, W+resid+store(w-2),
  reduce(w-1) on DVE, f(w,k2=2,3)/exp, ones(w-1), y(w-1), recip, ynorm.
- engines: ScalarE = phi/theta proj copies + all exps; DVE = g proj copies,
  all pools, A-tree reduce, reciprocal, gT casts, ynorm STT; PE = matmuls
  only (the neuronxcc backend rejects TensorTensor on the Pool engine, so
  GpSimd only does memset/iota framework work).
- PSUM: psf [128,1024]x2 (4 banks) + pss [128,512] (1) + psy [128,512] (1)
  + psW [128,1024] (2) = 8 banks.
"""

import numpy as np

import concourse.bass as bass
import concourse.tile as tile
from concourse import bacc, mybir
from concourse.bass_utils import run_bass_kernel_spmd
from concourse.masks import make_identity

F32 = mybir.dt.float32
BF16 = mybir.dt.bfloat16
FP16 = mybir.dt.float16
AX = mybir.AluOpType
AF = mybir.ActivationFunctionType

B, C, HW = 8, 256, 4096
CI = 128
NK = 1024
N_CORES = 8
BN_EPS = 1e-5

WIN = 512
NWIN = HW // WIN

# x chunk column widths per t-half (q0 split for an early first conv)
XCHUNKS = [512, 512, 1024, 1024, 1024]
XOFF = [0, 512, 1024, 2048, 3072]


def build_program():
    nc = bacc.Bacc("TRN2", target_bir_lowering=False, debug=False,
                   num_devices=N_CORES)

    x_d = nc.dram_tensor("x", [C, HW], FP16, kind="ExternalInput").ap()
    wq_d = nc.dram_tensor("wq", [128, 6 * 128], FP16, kind="ExternalInput").ap()
    wW_d = nc.dram_tensor("wW", [128, 2 * 128], BF16, kind="ExternalInput").ap()
    bpk_d = nc.dram_tensor("bpk", [128, 3], F32, kind="ExternalInput").ap()
    z_d = nc.dram_tensor("z", [C, HW], FP16, kind="ExternalOutput").ap()

    with tile.TileContext(nc) as tc:
        with (
            tc.tile_pool(name="const", bufs=1) as consts,
            tc.tile_pool(name="xs", bufs=1) as xs,
            tc.tile_pool(name="big", bufs=1) as big,
            tc.tile_pool(name="stage", bufs=2) as stage,
            tc.tile_pool(name="ppf", bufs=2, space="PSUM") as ppf,
            tc.tile_pool(name="pps", bufs=1, space="PSUM") as pps,
            tc.tile_pool(name="ppy", bufs=1, space="PSUM") as ppy,
            tc.tile_pool(name="ppw", bufs=1, space="PSUM") as ppw,
        ):
            ident = consts.tile([128, 128], F32, tag="ident", name="ident")
            make_identity(nc, ident)
            ident_h = consts.tile([128, 128], FP16, tag="identh",
                                  name="identh")
            make_identity(nc, ident_h)
            ones_mat = consts.tile([128, 512], BF16, tag="ones_mat",
                                   name="ones_mat")
            nc.vector.memset(ones_mat, 1.0)

            # ---------------- loads ----------------
            wq = consts.tile([128, 6, 128], FP16, tag="wq", name="wq")
            wW = consts.tile([128, 2, 128], BF16, tag="wW", name="wW")
            bpk = consts.tile([128, 3], F32, tag="bpk", name="bpk")
            biases = bpk[:, 0:3]

            x_t = [[xs.tile([128, XCHUNKS[i]], FP16, tag=f"x{t}{i}",
                            name=f"x{t}{i}") for i in range(5)]
                   for t in range(2)]

            wqf = wq.rearrange("p j c -> p (j c)")
            nc.sync.dma_start(out=wqf, in_=wq_d)
            nc.scalar.dma_start(out=bpk, in_=bpk_d)
            for i in range(5):
                nc.sync.dma_start(out=x_t[0][i],
                                  in_=x_d[0:128, XOFF[i]:XOFF[i] + XCHUNKS[i]])
                nc.scalar.dma_start(out=x_t[1][i],
                                    in_=x_d[128:256,
                                            XOFF[i]:XOFF[i] + XCHUNKS[i]])
            wWf = wW.rearrange("p j c -> p (j c)")
            nc.scalar.dma_start(out=wWf, in_=wW_d)

            # ---------------- PE p-state warmup ----------------
            # no data deps beyond the ones memset; keeps the PE streaming
            # through the DMA head so the first conv runs at full clock
            pwu = ppw.tile([128, 1024], F32, tag="psW", name="pwu")
            for _ in range(8):
                nc.tensor.matmul(pwu[:, 0:512], ones_mat[:, 0:128], ones_mat,
                                 start=True, stop=True)

            # ---------------- projections (+ window-0 front-run) ----------
            theta_r = big.tile([128, HW], FP16, tag="theta", name="theta")
            pf_phi = big.tile([128, 64, 64], FP16, tag="pf_phi", name="pf_phi")
            m1 = big.tile([128, 64, 32], FP16, tag="m1", name="m1")
            phi_r = big.tile([128, NK], FP16, tag="phi", name="phi")
            pf_g = big.tile([128, 64, 64], BF16, tag="pf_g", name="pf_g")
            m1g = big.tile([128, 64, 32], BF16, tag="m1g", name="m1g")
            g_pool = big.tile([128, NK], F32, tag="gpool", name="gpool")
            gT_r = big.tile([128, 8, 128], BF16, tag="gT", name="gT")

            pf_phi_f = pf_phi.rearrange("p h w -> p (h w)")
            pf_g_f = pf_g.rearrange("p h w -> p (h w)")

            a_tiles = [None] * NWIN

            def conv(widx, q, dst_f):
                # 1x1 conv chunk q of projection widx into psum, then the
                # bias-add copy to SBUF on ScalarE (phi/theta) or DVE (g)
                psc = ppf.tile([128, 1024], F32, tag="psf", name="psc")
                # matmul outputs are capped at one psum bank (512 f32 cols)
                pieces = ([(0, 0, 0), (1, 512, 0)] if q == 0 else
                          [(q + 1, 0, 0), (q + 1, 512, 512)])
                for t in range(2):
                    for (ci, off, src_off) in pieces:
                        nc.tensor.matmul(psc[:, off:off + 512],
                                         wq[:, 2 * widx + t, :],
                                         x_t[t][ci][:, src_off:src_off + 512],
                                         start=(t == 0), stop=(t == 1))
                sl = slice(q * 1024, (q + 1) * 1024)
                if widx < 2:
                    nc.scalar.activation(out=dst_f[:, sl], in_=psc,
                                         func=AF.Identity,
                                         bias=biases[:, widx:widx + 1])
                else:
                    nc.vector.tensor_scalar_add(dst_f[:, sl], psc,
                                                biases[:, widx:widx + 1])

            def pool1(srcf, dst_m1, q, eng):
                a = srcf.rearrange("p h (w2 two) -> p h w2 two", two=2)
                eng.tensor_max(dst_m1[:, 16 * q:16 * (q + 1), :],
                               a[:, 16 * q:16 * (q + 1), :, 0],
                               a[:, 16 * q:16 * (q + 1), :, 1])

            def pool2_q(src_m1, dst, q, eng):
                # rows h2 in [8q, 8q+8) -> dst cols [256q, 256q+256)
                b_ = src_m1.rearrange("p (h2 two) w -> p h2 two w", two=2)
                eng.tensor_max(
                    dst[:, 256 * q:256 * (q + 1)].rearrange(
                        "p (h w) -> p h w", h=8),
                    b_[:, 8 * q:8 * (q + 1), 0, :],
                    b_[:, 8 * q:8 * (q + 1), 1, :])

            def f_pair(w, k2):
                # psf[k-subchunk j, n] for window w, k-chunks (2k2, 2k2+1),
                # then exp -> A[w] bf16
                if a_tiles[w] is None:
                    a_tiles[w] = big.tile([128, 8, WIN], BF16, tag="A",
                                          name=f"A{w}", bufs=3)
                a_t = a_tiles[w]
                sl = slice(w * WIN, (w + 1) * WIN)
                psf = ppf.tile([128, 2 * WIN], F32, tag="psf", name="psf")
                for j in range(2):
                    nc.tensor.matmul(
                        psf[:, j * WIN:(j + 1) * WIN],
                        phi_r[:, (2 * k2 + j) * 128:(2 * k2 + j + 1) * 128],
                        theta_r[:, sl], start=True, stop=True)
                nc.scalar.activation(
                    out=a_t.rearrange("p k n -> p (k n)")
                    [:, 2 * k2 * WIN:(2 * k2 + 2) * WIN],
                    in_=psf, func=AF.Exp)

            for q in range(4):
                conv(1, q, pf_phi_f)           # phi
                conv(0, q, theta_r)            # theta
                conv(2, q, pf_g_f)             # g (copy on DVE)
                pool1(pf_phi, m1, q, nc.vector)
                pool2_q(m1, phi_r, q, nc.vector)
                pool1(pf_g, m1g, q, nc.vector)
                f_pair(0, q)                   # window-0 front-run

            # g 2x2 pool tail + gT transposes (interleaved with produce(1))
            b_ = m1g.rearrange("p (h2 two) w -> p h2 two w", two=2)
            nc.vector.tensor_max(
                g_pool.rearrange("p (h w) -> p h w", h=32),
                b_[:, :, 0, :], b_[:, :, 1, :])

            g_bf = g_pool  # bf16 already

            for k in range(8):
                ptr = ppy.tile([128, WIN], F32, tag="psy", name="ptr")
                nc.tensor.transpose(ptr[:, :128],
                                    g_bf[:, k * 128:(k + 1) * 128], ident)
                nc.vector.tensor_copy(gT_r[:, k, :], ptr[:, :128])
                if k % 2 == 1:
                    f_pair(1, k // 2)

            # ---------------- attention pipeline ----------------
            r4s = [None] * NWIN
            r3s = [None] * NWIN
            y_tiles = [None] * NWIN
            rbcs = [None] * NWIN

            def reduce_tree(w):
                # A [128, 8, 512] bf16 -> r3 [128, 512] partial k-sums (DVE)
                a_t = a_tiles[w]
                r4 = big.tile([128, 4, WIN], BF16, tag="r4", name="r4",
                              bufs=2)
                r2 = big.tile([128, 2, WIN], BF16, tag="r2", name="r2",
                              bufs=2)
                r3 = big.tile([128, WIN], BF16, tag="r3", name="r3", bufs=2)
                nc.vector.tensor_add(r4, a_t[:, 0:4, :], a_t[:, 4:8, :])
                nc.vector.tensor_add(r2, r4[:, 0:2, :], r4[:, 2:4, :])
                nc.vector.tensor_add(r3, r2[:, 0, :], r2[:, 1, :])
                r4s[w] = r4
                r3s[w] = r3

            def consume_a(w):
                # ones matmul: every psum row = sum_k A[k, n] (broadcast for
                # free); then the unnormalized y accumulation
                pss = pps.tile([128, WIN], F32, tag="pss", name="pss")
                nc.tensor.matmul(pss, ones_mat[:, 0:128], r3s[w],
                                 start=True, stop=True)
                a_t = a_tiles[w]
                psy = ppy.tile([128, WIN], F32, tag="psy", name="psy")
                for k in range(8):
                    nc.tensor.matmul(psy, gT_r[:, k, :], a_t[:, k, :],
                                     start=(k == 0), stop=(k == 7))
                rbc = stage.tile([128, WIN], F32, tag="rbc", name="rbc")
                nc.vector.reciprocal_approx_fast(out=rbc, in_=pss)
                rbcs[w] = rbc
                y_r = stage.tile([128, WIN], BF16, tag="yr", name="yr")
                y_tiles[w] = y_r
                nc.vector.scalar_tensor_tensor(out=y_r, in0=psy, scalar=1.0,
                                               in1=rbc, op0=AX.mult,
                                               op1=AX.mult)

            def consume_w(w):
                # z = WW' @ y_norm + x'  accumulated in psum (residual via
                # identity matmul; BN scale/bias host-folded), then the
                # store DMAs straight from PSUM
                psW = ppw.tile([128, 1024], F32, tag="psW", name="psW")
                base = w * WIN
                ci = 0
                while XOFF[ci] + XCHUNKS[ci] <= base:
                    ci += 1
                for o in range(2):
                    osl = slice(o * WIN, (o + 1) * WIN)
                    nc.tensor.matmul(psW[:, osl], wW[:, o, :], y_tiles[w],
                                     start=True, stop=False)
                    xsl = x_t[o][ci][:, base - XOFF[ci]:
                                     base - XOFF[ci] + WIN]
                    nc.tensor.matmul(psW[:, osl], ident_h, xsl,
                                     start=False, stop=True)
                zs = stage.tile([128, 1024], FP16, tag="zs", name="zs",
                                bufs=3)
                nc.vector.tensor_copy(zs, psW)
                for o in range(2):
                    nc.sync.dma_start(
                        out=z_d[o * 128:(o + 1) * 128, base:base + WIN],
                        in_=zs[:, o * WIN:(o + 1) * WIN])

            # prologue: w0 reduced; w0 consumed; pipeline w=2..7
            reduce_tree(0)
            consume_a(0)
            for w in range(2, NWIN):
                f_pair(w, 0)
                f_pair(w, 1)
                consume_w(w - 2)
                reduce_tree(w - 1)
                f_pair(w, 2)
                f_pair(w, 3)
                consume_a(w - 1)
            consume_w(NWIN - 2)
            reduce_tree(NWIN - 1)
            consume_a(NWIN - 1)
            consume_w(NWIN - 1)
    nc.compile()
    return nc


_nc_cache = None


def _get_nc():
    global _nc_cache
    if _nc_cache is None:
        _nc_cache = build_program()
    return _nc_cache


def run(inputs, trace=False, **kw):
    lc = np.asarray(inputs["lc"], dtype=np.float32)
    fuse = np.asarray(inputs["fuse"], dtype=np.float32)
    gc = np.asarray(inputs["gc"], dtype=np.float32)

    inv = np.asarray(inputs["bn_gamma"], np.float32) / np.sqrt(
        np.asarray(inputs["bn_var"], np.float32) + BN_EPS)
    bias2 = ((np.asarray(inputs["W_b"], np.float32)
              - np.asarray(inputs["bn_mean"], np.float32)) * inv
             + np.asarray(inputs["bn_beta"], np.float32))

    import ml_dtypes
    # x' = x + bias2 rides the residual identity-matmul; the conv biases are
    # adjusted by -W_proj @ bias2 so the projections still see plain x
    wq = np.empty((128, 6 * 128), np.float32)
    bpk = np.empty((128, 3), np.float32)
    for i, (wn, bn) in enumerate((("theta_w", "theta_b"), ("phi_w", "phi_b"),
                                  ("g_w", "g_b"))):
        wmat = np.asarray(inputs[wn], np.float32)          # [CI, C]
        # wq[p, (2i+t)*128 + c] = W_i.T[t*128+p, c] (stationary [cin, cout])
        wt = wmat.T.reshape(2, 128, 128)
        wq[:, 2 * i * 128:(2 * i + 2) * 128] = \
            wt.transpose(1, 0, 2).reshape(128, 256)
        bpk[:, i] = np.asarray(inputs[bn], np.float32) - wmat @ bias2
    wq = wq.astype(np.float16)
    # BN scale folded into the W-conv weights
    wW = (np.asarray(inputs["W_w"], np.float32) * inv[:, None]) \
        .T.reshape(128, 256).astype(ml_dtypes.bfloat16)
    common = {"wq": wq, "wW": wW, "bpk": bpk}
    in_maps = []
    for b in range(B):
        m = dict(common)
        m["x"] = np.ascontiguousarray(
            (fuse[b].reshape(C, HW) + bias2[:, None]).astype(np.float16))
        in_maps.append(m)

    nc = _get_nc()
    res = run_bass_kernel_spmd(nc, in_maps, core_ids=list(range(N_CORES)),
                               trace=trace, **kw)

    out = np.empty((B, 3 * C, 64, 64), dtype=np.float32)
    out[:, :C] = lc
    for b in range(B):
        out[b, C:2 * C] = np.asarray(res.results[b]["z"], np.float32) \
            .reshape(C, 64, 64)
    out[:, 2 * C:] = gc
    return out, res


def kernel(**inputs) -> np.ndarray:
    out, _ = run(inputs, trace=False)
    return out


# revision 27
# speedup vs baseline: 1.0510x; 1.0510x over previous
"""Trainium2 Bass kernel for the non-local attention block (nn_ASM_5196910428634).

8 NeuronCores, data-parallel over batch (1 element per core).  Per core:
  x = fuse[b] as [C=256, HW=4096] (host-cast to fp16: halves the DMA-in
      stream that paces the conv phase; residual in fp16 costs ~2e-3 abs)
  theta = Wt @ x + bt                [128, 4096]   (fp16 conv, fp16 out)
  phi   = pool2(Wp @ x + bp)         [128, 1024]   (fp16 conv, fp16 out)
  g     = pool2(Wg @ x + bg)         [128, 1024]   (fp16 conv, bf16 out)
  fT[k, n] = sum_ci phi[ci, k] theta[ci, n]        (fp16 matmul, k=1024)
  A = exp(fT)  -> bf16               (softmax w/o max-subtraction: |f| << 88,
                                      and bf16 carries the fp32 exponent range)
  sums[n]: 3-level bf16 pairwise tree over A's 8 k-chunks on DVE, then ONE
           all-ones 128x128 stationary matmul for the partition reduction
           (whose output rows ARE the broadcast) - replaces the 8 ones
           matmuls/window of the previous version (-12us PE)
  yT[c, n] = sum_k gT[k, c] A[k, n]  (bf16 matmul, unnormalized)
  y_norm = yT * (1/sums)             (reciprocal_approx_fast on DVE, the
                                      normalize STT on the GpSimd engine)
  z = WW' @ y_norm + x'              (bf16 W conv with the BN scale folded
                                      into WW' on host; x' = x + bias2 is
                                      host-folded with conv biases adjusted
                                      by -W_proj @ bias2, so the residual +
                                      BN bias is ONE identity matmul into
                                      the same psum, and z DMAs straight
                                      from PSUM - no vector-engine tail)
Host assembles out = concat([lc, z, gc], axis=1) (pure pass-through channels).

Schedule notes:
- ~8 dummy 512-row matmuls right after the const memsets keep the PE
  executing through the DMA head so its p-state ramps to 2.4GHz before the
  first conv (cold matmuls otherwise run 2x slow for the first ~3us).
- x + weights ride the sync/scalar HWDGE queues (t=0 rows on sync, t=1 on
  scalar) in conv consumption order; the stream is HBM-bandwidth-bound.
- phi's 2x2 maxpool is split per conv chunk q so window 0's f matmuls and
  exps run interleaved WITH the conv phase; the ScalarE exp stream (the
  steady-state pacer at ~4.5us/window) starts ~13us in.
- steady-state loop per window w: f(w,k2=0,1)/exp, W+resid+store(w-2),
  reduce(w-1) on DVE, f(w,k2=2,3)/exp, ones(w-1), y(w-1), recip, ynorm.
- engines: ScalarE = phi/theta proj copies + all exps; DVE = g proj copies,
  all pools, A-tree reduce, reciprocal, gT casts, ynorm STT; PE = matmuls
  only (the neuronxcc backend rejects TensorTensor on the Pool engine, so
  GpSimd only does memset/iota framework work).
- PSUM: psf [128,1024]x2 (4 banks) + pss [128,512] (1) + psy [128,512] (1)
  + psW [128,1024] (2) = 8 banks.
"""

import numpy as np

import concourse.bass as bass
import concourse.tile as tile
from concourse import bacc, mybir
from concourse.bass_utils import run_bass_kernel_spmd
from concourse.masks import make_identity

F32 = mybir.dt.float32
BF16 = mybir.dt.bfloat16
FP16 = mybir.dt.float16
AX = mybir.AluOpType
AF = mybir.ActivationFunctionType

B, C, HW = 8, 256, 4096
CI = 128
NK = 1024
N_CORES = 8
BN_EPS = 1e-5

WIN = 512
NWIN = HW // WIN

# x chunk column widths per t-half (q0 split for an early first conv)
XCHUNKS = [512, 512, 1024, 1024, 1024]
XOFF = [0, 512, 1024, 2048, 3072]


def build_program():
    nc = bacc.Bacc("TRN2", target_bir_lowering=False, debug=False,
                   num_devices=N_CORES)

    x_d = nc.dram_tensor("x", [C, HW], FP16, kind="ExternalInput").ap()
    wq_d = nc.dram_tensor("wq", [128, 6 * 128], FP16, kind="ExternalInput").ap()
    wW_d = nc.dram_tensor("wW", [128, 2 * 128], BF16, kind="ExternalInput").ap()
    bpk_d = nc.dram_tensor("bpk", [128, 3], F32, kind="ExternalInput").ap()
    z_d = nc.dram_tensor("z", [C, HW], FP16, kind="ExternalOutput").ap()

    with tile.TileContext(nc) as tc:
        with (
            tc.tile_pool(name="const", bufs=1) as consts,
            tc.tile_pool(name="xs", bufs=1) as xs,
            tc.tile_pool(name="big", bufs=1) as big,
            tc.tile_pool(name="stage", bufs=2) as stage,
            tc.tile_pool(name="ppf", bufs=2, space="PSUM") as ppf,
            tc.tile_pool(name="pps", bufs=1, space="PSUM") as pps,
            tc.tile_pool(name="ppy", bufs=1, space="PSUM") as ppy,
            tc.tile_pool(name="ppw", bufs=1, space="PSUM") as ppw,
        ):
            ones_mat = consts.tile([128, 512], BF16, tag="ones_mat",
                                   name="ones_mat")
            nc.vector.memset(ones_mat, 1.0)
            ident = consts.tile([128, 128], F32, tag="ident", name="ident")
            make_identity(nc, ident)

            # ---------------- loads ----------------
            wq = consts.tile([128, 6, 128], FP16, tag="wq", name="wq")
            wW = consts.tile([128, 2, 128], BF16, tag="wW", name="wW")
            bpk = consts.tile([128, 3], F32, tag="bpk", name="bpk")
            biases = bpk[:, 0:3]

            x_t = [[xs.tile([128, XCHUNKS[i]], FP16, tag=f"x{t}{i}",
                            name=f"x{t}{i}") for i in range(5)]
                   for t in range(2)]

            # wq chunk order (host-packed): phi t0/t1, theta t0/t1, g t0/t1;
            # split the weight DMA so the first conv's stationary lands first
            wqf = wq.rearrange("p j c -> p (j c)")

            def xload(t, i, eng):
                eng.dma_start(out=x_t[t][i],
                              in_=x_d[t * 128:(t + 1) * 128,
                                      XOFF[i]:XOFF[i] + XCHUNKS[i]])

            nc.sync.dma_start(out=wqf[:, 0:256], in_=wq_d[:, 0:256])   # phi
            nc.scalar.dma_start(out=bpk, in_=bpk_d)
            xload(0, 0, nc.sync)
            xload(1, 0, nc.scalar)
            nc.sync.dma_start(out=wqf[:, 256:512], in_=wq_d[:, 256:512])
            nc.scalar.dma_start(out=wqf[:, 512:768], in_=wq_d[:, 512:768])
            xload(0, 1, nc.sync)
            xload(1, 1, nc.scalar)
            for i in range(2, 5):
                xload(0, i, nc.sync)
                xload(1, i, nc.scalar)
            wWf = wW.rearrange("p j c -> p (j c)")
            nc.scalar.dma_start(out=wWf, in_=wW_d)

            # ---------------- PE p-state warmup ----------------
            # no data deps beyond the ones memset; keeps the PE streaming
            # through the DMA head so the first conv runs at full clock
            pwu = ppw.tile([128, 1024], F32, tag="psW", name="pwu")
            for _ in range(8):
                nc.tensor.matmul(pwu[:, 0:512], ones_mat[:, 0:128], ones_mat,
                                 start=True, stop=True)

            # ---------------- projections (+ window-0 front-run) ----------
            theta_r = big.tile([128, HW], FP16, tag="theta", name="theta")
            pf_phi = big.tile([128, 64, 64], FP16, tag="pf_phi", name="pf_phi")
            m1 = big.tile([128, 64, 32], FP16, tag="m1", name="m1")
            phi_r = big.tile([128, NK], FP16, tag="phi", name="phi")
            pf_g = big.tile([128, 64, 64], BF16, tag="pf_g", name="pf_g")
            m1g = big.tile([128, 64, 32], BF16, tag="m1g", name="m1g")
            g_pool = big.tile([128, NK], F32, tag="gpool", name="gpool")
            gT_r = big.tile([128, 8, 128], BF16, tag="gT", name="gT")

            pf_phi_f = pf_phi.rearrange("p h w -> p (h w)")
            pf_g_f = pf_g.rearrange("p h w -> p (h w)")

            a_tiles = [None] * NWIN

            def conv(widx, q, dst_f):
                # 1x1 conv chunk q of projection widx (0=phi, 1=theta, 2=g)
                # into psum, then the bias-add copy to SBUF on ScalarE
                # (phi/theta) or DVE (g)
                psc = ppf.tile([128, 1024], F32, tag="psf", name="psc")
                # matmul outputs are capped at one psum bank (512 f32 cols)
                pieces = ([(0, 0, 0), (1, 512, 0)] if q == 0 else
                          [(q + 1, 0, 0), (q + 1, 512, 512)])
                for t in range(2):
                    for (ci, off, src_off) in pieces:
                        nc.tensor.matmul(psc[:, off:off + 512],
                                         wq[:, 2 * widx + t, :],
                                         x_t[t][ci][:, src_off:src_off + 512],
                                         start=(t == 0), stop=(t == 1))
                sl = slice(q * 1024, (q + 1) * 1024)
                if widx < 2:
                    nc.scalar.activation(out=dst_f[:, sl], in_=psc,
                                         func=AF.Identity,
                                         bias=biases[:, widx:widx + 1])
                else:
                    nc.vector.tensor_scalar_add(dst_f[:, sl], psc,
                                                biases[:, widx:widx + 1])

            def pool1(srcf, dst_m1, q, eng):
                a = srcf.rearrange("p h (w2 two) -> p h w2 two", two=2)
                eng.tensor_max(dst_m1[:, 16 * q:16 * (q + 1), :],
                               a[:, 16 * q:16 * (q + 1), :, 0],
                               a[:, 16 * q:16 * (q + 1), :, 1])

            def pool2_q(src_m1, dst, q, eng):
                # rows h2 in [8q, 8q+8) -> dst cols [256q, 256q+256)
                b_ = src_m1.rearrange("p (h2 two) w -> p h2 two w", two=2)
                eng.tensor_max(
                    dst[:, 256 * q:256 * (q + 1)].rearrange(
                        "p (h w) -> p h w", h=8),
                    b_[:, 8 * q:8 * (q + 1), 0, :],
                    b_[:, 8 * q:8 * (q + 1), 1, :])

            def f_pair(w, k2):
                # psf[k-subchunk j, n] for window w, k-chunks (2k2, 2k2+1),
                # then exp -> A[w] bf16
                if a_tiles[w] is None:
                    a_tiles[w] = big.tile([128, 8, WIN], BF16, tag="A",
                                          name=f"A{w}", bufs=3)
                a_t = a_tiles[w]
                sl = slice(w * WIN, (w + 1) * WIN)
                psf = ppf.tile([128, 2 * WIN], F32, tag="psf", name="psf")
                for j in range(2):
                    nc.tensor.matmul(
                        psf[:, j * WIN:(j + 1) * WIN],
                        phi_r[:, (2 * k2 + j) * 128:(2 * k2 + j + 1) * 128],
                        theta_r[:, sl], start=True, stop=True)
                nc.scalar.activation(
                    out=a_t.rearrange("p k n -> p (k n)")
                    [:, 2 * k2 * WIN:(2 * k2 + 2) * WIN],
                    in_=psf, func=AF.Exp)

            for q in range(4):
                conv(0, q, pf_phi_f)           # phi
                conv(1, q, theta_r)            # theta
                conv(2, q, pf_g_f)             # g (copy on DVE)
                pool1(pf_phi, m1, q, nc.vector)
                pool2_q(m1, phi_r, q, nc.vector)
                pool1(pf_g, m1g, q, nc.vector)
                if q > 0:
                    # window-0 front-run, one q behind so the PE never waits
                    # on the copy->pool1->pool2 chain of the current q
                    f_pair(0, q - 1)
            f_pair(0, 3)

            # g 2x2 pool tail + gT transposes (interleaved with produce(1))
            b_ = m1g.rearrange("p (h2 two) w -> p h2 two w", two=2)
            nc.vector.tensor_max(
                g_pool.rearrange("p (h w) -> p h w", h=32),
                b_[:, :, 0, :], b_[:, :, 1, :])

            g_bf = g_pool  # bf16 already

            for k in range(8):
                ptr = ppy.tile([128, WIN], F32, tag="psy", name="ptr")
                nc.tensor.transpose(ptr[:, :128],
                                    g_bf[:, k * 128:(k + 1) * 128], ident)
                nc.vector.tensor_copy(gT_r[:, k, :], ptr[:, :128])
                if k % 2 == 1:
                    f_pair(1, k // 2)

            # ---------------- attention pipeline ----------------
            rb1s = [None] * NWIN
            r3s = [None] * NWIN
            y_tiles = [None] * NWIN

            def reduce_a(w):
                # first half of the k-sum tree: needs only exps (w,0),(w,1)
                a_t = a_tiles[w]
                rb1 = big.tile([128, 2, WIN], BF16, tag="rb1", name="rb1",
                               bufs=2)
                nc.vector.tensor_add(rb1, a_t[:, 0:2, :], a_t[:, 2:4, :])
                rb1s[w] = rb1

            def reduce_b(w):
                # second half + combine -> r3 [128, 512] partial k-sums
                a_t = a_tiles[w]
                rb2 = big.tile([128, 2, WIN], BF16, tag="rb2", name="rb2",
                               bufs=2)
                rc = big.tile([128, 2, WIN], BF16, tag="rc", name="rc",
                              bufs=2)
                r3 = big.tile([128, WIN], BF16, tag="r3", name="r3", bufs=2)
                nc.vector.tensor_add(rb2, a_t[:, 4:6, :], a_t[:, 6:8, :])
                nc.vector.tensor_add(rc, rb1s[w], rb2)
                nc.vector.tensor_add(r3, rc[:, 0, :], rc[:, 1, :])
                r3s[w] = r3

            def consume_a(w):
                # y first (it only needs A), then the ones matmul: every pss
                # row = sum_k A[k, n] (partition-broadcast for free)
                a_t = a_tiles[w]
                psy = ppy.tile([128, WIN], F32, tag="psy", name="psy")
                for k in range(8):
                    nc.tensor.matmul(psy, gT_r[:, k, :], a_t[:, k, :],
                                     start=(k == 0), stop=(k == 7))
                pss = pps.tile([128, WIN], F32, tag="pss", name="pss")
                nc.tensor.matmul(pss, ones_mat[:, 0:128], r3s[w],
                                 start=True, stop=True)
                rbc = stage.tile([128, WIN], F32, tag="rbc", name="rbc")
                nc.vector.reciprocal_approx_fast(out=rbc, in_=pss)
                y_r = stage.tile([128, WIN], BF16, tag="yr", name="yr")
                y_tiles[w] = y_r
                nc.vector.scalar_tensor_tensor(out=y_r, in0=psy, scalar=1.0,
                                               in1=rbc, op0=AX.mult,
                                               op1=AX.mult)

            def consume_w(w):
                # z = WW' @ y_norm (BN scale host-folded into wW), then
                # zs = psW + x' on DVE (residual + host-folded BN bias),
                # stores from SBUF fp16
                psW = ppw.tile([128, 1024], F32, tag="psW", name="psW")
                base = w * WIN
                ci = 0
                while XOFF[ci] + XCHUNKS[ci] <= base:
                    ci += 1
                for o in range(2):
                    nc.tensor.matmul(psW[:, o * WIN:(o + 1) * WIN],
                                     wW[:, o, :], y_tiles[w],
                                     start=True, stop=True)
                zs = stage.tile([128, 1024], FP16, tag="zs", name="zs",
                                bufs=3)
                for o in range(2):
                    xsl = x_t[o][ci][:, base - XOFF[ci]:
                                     base - XOFF[ci] + WIN]
                    nc.vector.tensor_add(zs[:, o * WIN:(o + 1) * WIN],
                                         psW[:, o * WIN:(o + 1) * WIN], xsl)
                for o in range(2):
                    nc.sync.dma_start(
                        out=z_d[o * 128:(o + 1) * 128, base:base + WIN],
                        in_=zs[:, o * WIN:(o + 1) * WIN])

            # prologue for w0/w1, then the steady loop w=2..7
            reduce_a(0)
            reduce_b(0)
            consume_a(0)
            reduce_a(1)
            for w in range(2, NWIN):
                f_pair(w, 0)
                f_pair(w, 1)
                reduce_b(w - 1)
                consume_w(w - 2)
                f_pair(w, 2)
                f_pair(w, 3)
                reduce_a(w)
                consume_a(w - 1)
            consume_w(NWIN - 2)
            reduce_b(NWIN - 1)
            consume_a(NWIN - 1)
            consume_w(NWIN - 1)
    nc.compile()
    return nc


_nc_cache = None


def _get_nc():
    global _nc_cache
    if _nc_cache is None:
        _nc_cache = build_program()
    return _nc_cache


def run(inputs, trace=False, **kw):
    lc = np.asarray(inputs["lc"], dtype=np.float32)
    fuse = np.asarray(inputs["fuse"], dtype=np.float32)
    gc = np.asarray(inputs["gc"], dtype=np.float32)

    inv = np.asarray(inputs["bn_gamma"], np.float32) / np.sqrt(
        np.asarray(inputs["bn_var"], np.float32) + BN_EPS)
    bias2 = ((np.asarray(inputs["W_b"], np.float32)
              - np.asarray(inputs["bn_mean"], np.float32)) * inv
             + np.asarray(inputs["bn_beta"], np.float32))

    import ml_dtypes
    # x' = x + bias2 rides the residual identity-matmul; the conv biases are
    # adjusted by -W_proj @ bias2 so the projections still see plain x
    wq = np.empty((128, 6 * 128), np.float32)
    bpk = np.empty((128, 3), np.float32)
    for i, (wn, bn) in enumerate((("phi_w", "phi_b"), ("theta_w", "theta_b"),
                                  ("g_w", "g_b"))):
        wmat = np.asarray(inputs[wn], np.float32)          # [CI, C]
        # wq[p, (2i+t)*128 + c] = W_i.T[t*128+p, c] (stationary [cin, cout])
        wt = wmat.T.reshape(2, 128, 128)
        wq[:, 2 * i * 128:(2 * i + 2) * 128] = \
            wt.transpose(1, 0, 2).reshape(128, 256)
        bpk[:, i] = np.asarray(inputs[bn], np.float32) - wmat @ bias2
    wq = wq.astype(np.float16)
    # BN scale folded into the W-conv weights
    wW = (np.asarray(inputs["W_w"], np.float32) * inv[:, None]) \
        .T.reshape(128, 256).astype(ml_dtypes.bfloat16)
    common = {"wq": wq, "wW": wW, "bpk": bpk}
    in_maps = []
    for b in range(B):
        m = dict(common)
        m["x"] = np.ascontiguousarray(
            (fuse[b].reshape(C, HW) + bias2[:, None]).astype(np.float16))
        in_maps.append(m)

    nc = _get_nc()
    res = run_bass_kernel_spmd(nc, in_maps, core_ids=list(range(N_CORES)),
                               trace=trace, **kw)

    out = np.empty((B, 3 * C, 64, 64), dtype=np.float32)
    out[:, :C] = lc
    for b in range(B):
        out[b, C:2 * C] = np.asarray(res.results[b]["z"], np.float32) \
            .reshape(C, 64, 64)
    out[:, 2 * C:] = gc
    return out, res


def kernel(**inputs) -> np.ndarray:
    out, _ = run(inputs, trace=False)
    return out


# revision 35
# speedup vs baseline: 1.0615x; 1.0100x over previous
"""Trainium2 Bass kernel for the non-local attention block (nn_ASM_5196910428634).

8 NeuronCores, data-parallel over batch (1 element per core).  Per core:
  x = fuse[b] as [C=256, HW=4096] (host-cast to fp16: halves the DMA-in
      stream that paces the conv phase; residual in fp16 costs ~2e-3 abs)
  theta = Wt @ x + bt                [128, 4096]   (fp16 conv, fp16 out)
  phi   = pool2(Wp @ x + bp)         [128, 1024]   (fp16 conv, fp16 out)
  g     = pool2(Wg @ x + bg)         [128, 1024]   (fp16 conv, bf16 out)
  fT[k, n] = sum_ci phi[ci, k] theta[ci, n]        (fp16 matmul, k=1024)
  A = exp(fT)  -> bf16               (softmax w/o max-subtraction: |f| << 88,
                                      and bf16 carries the fp32 exponent range)
  sums[n]: 3-level bf16 pairwise tree over A's 8 k-chunks on DVE, then ONE
           all-ones 128x128 stationary matmul for the partition reduction
           (whose output rows ARE the broadcast) - replaces the 8 ones
           matmuls/window of the previous version (-12us PE)
  yT[c, n] = sum_k gT[k, c] A[k, n]  (bf16 matmul, unnormalized)
  y_norm = yT * (1/sums)             (reciprocal_approx_fast on DVE, the
                                      normalize STT on the GpSimd engine)
  z = WW' @ y_norm + x'              (bf16 W conv with the BN scale folded
                                      into WW' on host; x' = x + bias2 is
                                      host-folded with conv biases adjusted
                                      by -W_proj @ bias2, so the residual +
                                      BN bias is ONE identity matmul into
                                      the same psum, and z DMAs straight
                                      from PSUM - no vector-engine tail)
Host assembles out = concat([lc, z, gc], axis=1) (pure pass-through channels).

Schedule notes:
- ~8 dummy 512-row matmuls right after the const memsets keep the PE
  executing through the DMA head so its p-state ramps to 2.4GHz before the
  first conv (cold matmuls otherwise run 2x slow for the first ~3us).
- x + weights ride the sync/scalar HWDGE queues (t=0 rows on sync, t=1 on
  scalar) in conv consumption order; the stream is HBM-bandwidth-bound.
- phi's 2x2 maxpool is split per conv chunk q so window 0's f matmuls and
  exps run interleaved WITH the conv phase; the ScalarE exp stream (the
  steady-state pacer at ~4.5us/window) starts ~13us in.
- steady-state loop per window w: f(w,k2=0,1)/exp, W+resid+store(w-2),
  reduce(w-1) on DVE, f(w,k2=2,3)/exp, ones(w-1), y(w-1), recip, ynorm.
- engines: ScalarE = phi/theta proj copies + all exps; DVE = g proj copies,
  all pools, A-tree reduce, reciprocal, gT casts, ynorm STT; PE = matmuls
  only (the neuronxcc backend rejects TensorTensor on the Pool engine, so
  GpSimd only does memset/iota framework work).
- PSUM: psf [128,1024]x2 (4 banks) + pss [128,512] (1) + psy [128,512] (1)
  + psW [128,1024] (2) = 8 banks.
"""

import numpy as np

import concourse.bass as bass
import concourse.tile as tile
from concourse import bacc, mybir
from concourse.bass_utils import run_bass_kernel_spmd
from concourse.masks import make_identity

F32 = mybir.dt.float32
BF16 = mybir.dt.bfloat16
FP16 = mybir.dt.float16
AX = mybir.AluOpType
AF = mybir.ActivationFunctionType

B, C, HW = 8, 256, 4096
CI = 128
NK = 1024
N_CORES = 8
BN_EPS = 1e-5

# attention windows (start, width); the last 512 block is split in two so
# the post-last-exp drain chain (reduce->ones->recip->ynorm->W->zs->store)
# runs at half width
WINS = [(i * 512, 512) for i in range(7)] + [(3584, 256), (3840, 256)]
NW = len(WINS)

# x chunk column widths per t-half (q0 split for an early first conv)
XCHUNKS = [512, 512, 1024, 1024, 1024]
XOFF = [0, 512, 1024, 2048, 3072]


def build_program():
    nc = bacc.Bacc("TRN2", target_bir_lowering=False, debug=False,
                   num_devices=N_CORES)

    x_d = nc.dram_tensor("x", [C, HW], FP16, kind="ExternalInput").ap()
    wq_d = nc.dram_tensor("wq", [128, 6 * 128], FP16, kind="ExternalInput").ap()
    wW_d = nc.dram_tensor("wW", [128, 2 * 128], BF16, kind="ExternalInput").ap()
    bpk_d = nc.dram_tensor("bpk", [128, 3], F32, kind="ExternalInput").ap()
    z_d = nc.dram_tensor("z", [C, HW], FP16, kind="ExternalOutput").ap()

    with tile.TileContext(nc) as tc:
        with (
            tc.tile_pool(name="const", bufs=1) as consts,
            tc.tile_pool(name="xs", bufs=1) as xs,
            tc.tile_pool(name="big", bufs=1) as big,
            tc.tile_pool(name="stage", bufs=2) as stage,
            tc.tile_pool(name="ppf", bufs=2, space="PSUM") as ppf,
            tc.tile_pool(name="pps", bufs=1, space="PSUM") as pps,
            tc.tile_pool(name="ppy", bufs=1, space="PSUM") as ppy,
            tc.tile_pool(name="ppw", bufs=1, space="PSUM") as ppw,
        ):
            # ones first (on gpsimd, whose queue is free earliest) so the PE
            # p-state warmup can start as soon as possible
            ones_mat = consts.tile([128, 512], BF16, tag="ones_mat",
                                   name="ones_mat")
            nc.gpsimd.memset(ones_mat, 1.0)
            ident = consts.tile([128, 128], F32, tag="ident", name="ident")
            make_identity(nc, ident)

            # ---------------- loads ----------------
            wq = consts.tile([128, 6, 128], FP16, tag="wq", name="wq")
            wW = consts.tile([128, 2, 128], BF16, tag="wW", name="wW")
            bpk = consts.tile([128, 3], F32, tag="bpk", name="bpk")
            biases = bpk[:, 0:3]

            x_t = [[xs.tile([128, XCHUNKS[i]], FP16, tag=f"x{t}{i}",
                            name=f"x{t}{i}") for i in range(5)]
                   for t in range(2)]

            # wq chunk order (host-packed): phi t0/t1, theta t0/t1, g t0/t1;
            # split the weight DMA so the first conv's stationary lands first
            wqf = wq.rearrange("p j c -> p (j c)")

            def xload(t, i, eng):
                eng.dma_start(out=x_t[t][i],
                              in_=x_d[t * 128:(t + 1) * 128,
                                      XOFF[i]:XOFF[i] + XCHUNKS[i]])

            nc.sync.dma_start(out=wqf[:, 0:256], in_=wq_d[:, 0:256])   # phi
            nc.scalar.dma_start(out=bpk, in_=bpk_d)
            xload(0, 0, nc.sync)
            xload(1, 0, nc.scalar)
            nc.sync.dma_start(out=wqf[:, 256:512], in_=wq_d[:, 256:512])
            nc.scalar.dma_start(out=wqf[:, 512:768], in_=wq_d[:, 512:768])
            xload(0, 1, nc.sync)
            xload(1, 1, nc.scalar)
            for i in range(2, 5):
                xload(0, i, nc.sync)
                xload(1, i, nc.scalar)
            wWf = wW.rearrange("p j c -> p (j c)")
            nc.scalar.dma_start(out=wWf, in_=wW_d)

            # ---------------- PE p-state warmup ----------------
            # no data deps beyond the ones memset; keeps the PE streaming
            # through the DMA head so the first conv runs at full clock
            pwu = ppw.tile([128, 1024], F32, tag="psW", name="pwu")
            for _ in range(8):
                nc.tensor.matmul(pwu[:, 0:512], ones_mat[:, 0:128], ones_mat,
                                 start=True, stop=True)

            # ---------------- projections (+ window-0 front-run) ----------
            theta_r = big.tile([128, HW], FP16, tag="theta", name="theta")
            pf_phi = big.tile([128, 64, 64], FP16, tag="pf_phi", name="pf_phi")
            m1 = big.tile([128, 64, 32], FP16, tag="m1", name="m1")
            phi_r = big.tile([128, NK], FP16, tag="phi", name="phi")
            pf_g = big.tile([128, 64, 64], BF16, tag="pf_g", name="pf_g")
            m1g = big.tile([128, 64, 32], BF16, tag="m1g", name="m1g")
            g_pool = big.tile([128, NK], F32, tag="gpool", name="gpool")
            gT_r = big.tile([128, 8, 128], BF16, tag="gT", name="gT")

            pf_phi_f = pf_phi.rearrange("p h w -> p (h w)")
            pf_g_f = pf_g.rearrange("p h w -> p (h w)")

            a_tiles = [None] * NW

            def conv(widx, q, dst_f):
                # 1x1 conv chunk q of projection widx (0=phi, 1=theta, 2=g)
                # into psum, then the bias-add copy to SBUF on ScalarE
                # (phi/theta) or DVE (g)
                psc = ppf.tile([128, 1024], F32, tag="psf", name="psc")
                # matmul outputs are capped at one psum bank (512 f32 cols)
                pieces = ([(0, 0, 0), (1, 512, 0)] if q == 0 else
                          [(q + 1, 0, 0), (q + 1, 512, 512)])
                for t in range(2):
                    for (ci, off, src_off) in pieces:
                        nc.tensor.matmul(psc[:, off:off + 512],
                                         wq[:, 2 * widx + t, :],
                                         x_t[t][ci][:, src_off:src_off + 512],
                                         start=(t == 0), stop=(t == 1))
                sl = slice(q * 1024, (q + 1) * 1024)
                if widx < 2 or q < 2:
                    # ScalarE has conv-phase headroom; DVE keeps the last
                    # two g copies (they land while ScalarE is exp-busy)
                    nc.scalar.activation(out=dst_f[:, sl], in_=psc,
                                         func=AF.Identity,
                                         bias=biases[:, widx:widx + 1])
                else:
                    nc.vector.tensor_scalar_add(dst_f[:, sl], psc,
                                                biases[:, widx:widx + 1])

            def pool1(srcf, dst_m1, q, eng):
                a = srcf.rearrange("p h (w2 two) -> p h w2 two", two=2)
                eng.tensor_max(dst_m1[:, 16 * q:16 * (q + 1), :],
                               a[:, 16 * q:16 * (q + 1), :, 0],
                               a[:, 16 * q:16 * (q + 1), :, 1])

            def pool2_q(src_m1, dst, q, eng):
                # rows h2 in [8q, 8q+8) -> dst cols [256q, 256q+256)
                b_ = src_m1.rearrange("p (h2 two) w -> p h2 two w", two=2)
                eng.tensor_max(
                    dst[:, 256 * q:256 * (q + 1)].rearrange(
                        "p (h w) -> p h w", h=8),
                    b_[:, 8 * q:8 * (q + 1), 0, :],
                    b_[:, 8 * q:8 * (q + 1), 1, :])

            def f_pair(w, k2):
                # psf[k-subchunk j, n] for window w, k-chunks (2k2, 2k2+1),
                # then exp -> A[w] bf16
                base, wd = WINS[w]
                if a_tiles[w] is None:
                    a_tiles[w] = big.tile([128, 8, 512], BF16, tag="A",
                                          name=f"A{w}", bufs=3)
                a_t = a_tiles[w]
                sl = slice(base, base + wd)
                psf = ppf.tile([128, 1024], F32, tag="psf", name="psf")
                for j in range(2):
                    nc.tensor.matmul(
                        psf[:, j * wd:(j + 1) * wd],
                        phi_r[:, (2 * k2 + j) * 128:(2 * k2 + j + 1) * 128],
                        theta_r[:, sl], start=True, stop=True)
                nc.scalar.activation(
                    out=a_t[:, 2 * k2:2 * k2 + 2, 0:wd],
                    in_=psf[:, 0:2 * wd], func=AF.Exp)

            for q in range(4):
                conv(0, q, pf_phi_f)           # phi
                conv(1, q, theta_r)            # theta
                conv(2, q, pf_g_f)             # g (copy on DVE)
                pool1(pf_phi, m1, q, nc.vector)
                pool2_q(m1, phi_r, q, nc.vector)
                pool1(pf_g, m1g, q, nc.vector)
                if q > 0:
                    # window-0 front-run, one q behind so the PE never waits
                    # on the copy->pool1->pool2 chain of the current q
                    f_pair(0, q - 1)
            f_pair(0, 3)

            # g 2x2 pool tail + gT transposes (interleaved with produce(1))
            b_ = m1g.rearrange("p (h2 two) w -> p h2 two w", two=2)
            nc.vector.tensor_max(
                g_pool.rearrange("p (h w) -> p h w", h=32),
                b_[:, :, 0, :], b_[:, :, 1, :])

            g_bf = g_pool  # bf16 already

            for k in range(8):
                ptr = ppy.tile([128, 512], F32, tag="psy", name="ptr")
                nc.tensor.transpose(ptr[:, :128],
                                    g_bf[:, k * 128:(k + 1) * 128], ident)
                nc.scalar.activation(out=gT_r[:, k, :], in_=ptr[:, :128],
                                     func=AF.Identity)
                if k % 2 == 1:
                    f_pair(1, k // 2)

            # ---------------- attention pipeline ----------------
            rb1s = [None] * NW
            rcs = [None] * NW
            y_tiles = [None] * NW

            def reduce_a(w):
                # first half of the k-sum tree: needs only exps (w,0),(w,1)
                wd = WINS[w][1]
                a_t = a_tiles[w]
                rb1 = big.tile([128, 2, 512], BF16, tag="rb1", name="rb1",
                               bufs=2)
                nc.vector.tensor_add(rb1[:, :, 0:wd], a_t[:, 0:2, 0:wd],
                                     a_t[:, 2:4, 0:wd])
                rb1s[w] = rb1

            def reduce_b(w):
                # second half + combine; the final pairwise step rides the
                # ones matmul (2 accumulating matmuls over rc)
                wd = WINS[w][1]
                a_t = a_tiles[w]
                rb2 = big.tile([128, 2, 512], BF16, tag="rb2", name="rb2",
                               bufs=2)
                rc = big.tile([128, 2, 512], BF16, tag="rc", name="rc",
                              bufs=2)
                nc.vector.tensor_add(rb2[:, :, 0:wd], a_t[:, 4:6, 0:wd],
                                     a_t[:, 6:8, 0:wd])
                nc.vector.tensor_add(rc[:, :, 0:wd], rb1s[w][:, :, 0:wd],
                                     rb2[:, :, 0:wd])
                rcs[w] = rc

            def consume_a(w, y_first=False):
                # ones matmuls: every pss row = sum_k A[k, n] (the partition
                # broadcast comes for free); steady state runs ones first so
                # recip/ynorm release early, the final window runs y first
                # so the PE overlaps the DVE reduce tail
                wd = WINS[w][1]
                a_t = a_tiles[w]

                def ones_mm():
                    pss = pps.tile([128, 512], F32, tag="pss", name="pss")
                    for j in range(2):
                        nc.tensor.matmul(pss[:, 0:wd], ones_mat[:, 0:128],
                                         rcs[w][:, j, 0:wd],
                                         start=(j == 0), stop=(j == 1))
                    return pss

                def y_mm():
                    psy = ppy.tile([128, 512], F32, tag="psy", name="psy")
                    for k in range(8):
                        nc.tensor.matmul(psy[:, 0:wd], gT_r[:, k, :],
                                         a_t[:, k, 0:wd],
                                         start=(k == 0), stop=(k == 7))
                    return psy

                if y_first:
                    psy = y_mm()
                    pss = ones_mm()
                else:
                    pss = ones_mm()
                    psy = y_mm()
                rbc = stage.tile([128, 512], F32, tag="rbc", name="rbc")
                nc.vector.reciprocal_approx_fast(out=rbc[:, 0:wd],
                                                 in_=pss[:, 0:wd])
                y_r = stage.tile([128, 512], BF16, tag="yr", name="yr")
                y_tiles[w] = y_r
                nc.vector.scalar_tensor_tensor(out=y_r[:, 0:wd],
                                               in0=psy[:, 0:wd], scalar=1.0,
                                               in1=rbc[:, 0:wd], op0=AX.mult,
                                               op1=AX.mult)

            def consume_w(w):
                # z = WW' @ y_norm (BN scale host-folded into wW), then
                # zs = psW + x' on DVE (residual + host-folded BN bias),
                # stores from SBUF fp16
                base, wd = WINS[w]
                psW = ppw.tile([128, 1024], F32, tag="psW", name="psW")
                ci = 0
                while XOFF[ci] + XCHUNKS[ci] <= base:
                    ci += 1
                for o in range(2):
                    nc.tensor.matmul(psW[:, o * 512:o * 512 + wd],
                                     wW[:, o, :], y_tiles[w][:, 0:wd],
                                     start=True, stop=True)
                zs = stage.tile([128, 1024], FP16, tag="zs", name="zs",
                                bufs=3)
                for o in range(2):
                    xsl = x_t[o][ci][:, base - XOFF[ci]:
                                     base - XOFF[ci] + wd]
                    nc.vector.tensor_add(zs[:, o * 512:o * 512 + wd],
                                         psW[:, o * 512:o * 512 + wd], xsl)
                for o in range(2):
                    nc.sync.dma_start(
                        out=z_d[o * 128:(o + 1) * 128, base:base + wd],
                        in_=zs[:, o * 512:o * 512 + wd])

            # prologue for w0/w1, then the steady loop w=2..NW-1
            reduce_a(0)
            reduce_b(0)
            consume_a(0)
            reduce_a(1)
            for w in range(2, NW):
                f_pair(w, 0)
                f_pair(w, 1)
                reduce_b(w - 1)
                consume_w(w - 2)
                f_pair(w, 2)
                f_pair(w, 3)
                consume_a(w - 1)
                reduce_a(w)
            consume_w(NW - 2)
            reduce_b(NW - 1)
            consume_a(NW - 1, y_first=True)
            consume_w(NW - 1)
    nc.compile()
    return nc


_nc_cache = None


def _get_nc():
    global _nc_cache
    if _nc_cache is None:
        _nc_cache = build_program()
    return _nc_cache


def run(inputs, trace=False, **kw):
    lc = np.asarray(inputs["lc"], dtype=np.float32)
    fuse = np.asarray(inputs["fuse"], dtype=np.float32)
    gc = np.asarray(inputs["gc"], dtype=np.float32)

    inv = np.asarray(inputs["bn_gamma"], np.float32) / np.sqrt(
        np.asarray(inputs["bn_var"], np.float32) + BN_EPS)
    bias2 = ((np.asarray(inputs["W_b"], np.float32)
              - np.asarray(inputs["bn_mean"], np.float32)) * inv
             + np.asarray(inputs["bn_beta"], np.float32))

    import ml_dtypes
    # x' = x + bias2 rides the residual identity-matmul; the conv biases are
    # adjusted by -W_proj @ bias2 so the projections still see plain x
    wq = np.empty((128, 6 * 128), np.float32)
    bpk = np.empty((128, 3), np.float32)
    for i, (wn, bn) in enumerate((("phi_w", "phi_b"), ("theta_w", "theta_b"),
                                  ("g_w", "g_b"))):
        wmat = np.asarray(inputs[wn], np.float32)          # [CI, C]
        # wq[p, (2i+t)*128 + c] = W_i.T[t*128+p, c] (stationary [cin, cout])
        wt = wmat.T.reshape(2, 128, 128)
        wq[:, 2 * i * 128:(2 * i + 2) * 128] = \
            wt.transpose(1, 0, 2).reshape(128, 256)
        bpk[:, i] = np.asarray(inputs[bn], np.float32) - wmat @ bias2
    wq = wq.astype(np.float16)
    # BN scale folded into the W-conv weights
    wW = (np.asarray(inputs["W_w"], np.float32) * inv[:, None]) \
        .T.reshape(128, 256).astype(ml_dtypes.bfloat16)
    common = {"wq": wq, "wW": wW, "bpk": bpk}
    in_maps = []
    for b in range(B):
        m = dict(common)
        m["x"] = np.ascontiguousarray(
            (fuse[b].reshape(C, HW) + bias2[:, None]).astype(np.float16))
        in_maps.append(m)

    nc = _get_nc()
    res = run_bass_kernel_spmd(nc, in_maps, core_ids=list(range(N_CORES)),
                               trace=trace, **kw)

    out = np.empty((B, 3 * C, 64, 64), dtype=np.float32)
    out[:, :C] = lc
    for b in range(B):
        out[b, C:2 * C] = np.asarray(res.results[b]["z"], np.float32) \
            .reshape(C, 64, 64)
    out[:, 2 * C:] = gc
    return out, res


def kernel(**inputs) -> np.ndarray:
    out, _ = run(inputs, trace=False)
    return out
